# revision 34
# baseline (speedup 1.0000x reference)
"""Trainium2 Bass kernel for nn_MultiHeadSelfAttentionL2 (B=8,S=1024,D=768,H=12,F=64).

Sharding: data-parallel over batch -- core i computes batch element i fully
(no collectives). Inside each core:

  phase 1: load x^T (bf16) + weights, project Q/K/V for all 12 heads.
           Q stored per-head as q_aug [66,1024]: rows 0-63 = q^T, row 64 = |q|^2,
           row 65 = ones. K stored as k_aug: rows 0-63 = -2*k^T, row 64 = ones,
           row 65 = |k|^2.  V stored as v_aug [128(t), 8(tchunk), 12(h), 65]:
           col 64 = ones (gives softmax denominator for free in the AV matmul).
  phase 2: per (head, 512-wide query chunk) "unit": ONE K=66 matmul per 128-key
           block produces the full squared L2 distance (qn + kn - 2qk) in PSUM;
           ACT Sqrt(scale=1/768) -> dist/sqrt(768); ACT Exp -> unnormalized att;
           AV matmul (M=65) gives head output + denominator row; reciprocal +
           K=1 broadcast matmul + DVE multiply applies the normalization.
           Units are processed in groups so sqrt/exp ACT-table loads amortize.
  phase 3: output linear from the transposed normalized head outputs
           (12 K=64 accumulating matmuls) + bias, DMA to HBM. First half is
           interleaved after the first 12 units (query chunk 0).

PSUM pools are shared across phases (single tag per pool) to avoid pool-boundary
serialization; phase-1 engine balance: PE matmuls, ACT squares, DVE drains/norm
staging, gpsimd memsets, DMA on both SP and ACT queues.
"""

import sys

import numpy as np

sys.path.insert(0, "/opt/trn_rl_repo")

import concourse.bass as bass
import concourse.tile as tile
from concourse import bacc, mybir
from concourse.bass_utils import run_bass_kernel_spmd

import ml_dtypes

B, S, D, H, F = 8, 1024, 768, 12, 64
O = H * F  # 768
NCORES = 8
BF = mybir.dt.bfloat16
F32 = mybir.dt.float32
AF = mybir.ActivationFunctionType

GROUP = 8    # units per sqrt/exp batch (ACT table amortization)
REPEAT = 1   # debug: repeat compute phases to measure marginal NEFF time


def _build(tc, xt, wqkt, wvt, wot, bo, out):
    from contextlib import ExitStack

    nc = tc.nc
    with ExitStack() as ctx:
        # shared PSUM pools (single tag each -> fixed bank budget, no barriers)
        pA = ctx.enter_context(tc.tile_pool(name="pA", bufs=2, space="PSUM"))
        pB = ctx.enter_context(tc.tile_pool(name="pB", bufs=2, space="PSUM"))
        pC = ctx.enter_context(tc.tile_pool(name="pC", bufs=2, space="PSUM"))

        persist = ctx.enter_context(tc.tile_pool(name="persist", bufs=1))
        q_aug = [persist.tile([66, S], BF, tag=f"qa{h}", name=f"qa{h}") for h in range(H)]
        k_aug = [persist.tile([66, S], BF, tag=f"ka{h}", name=f"ka{h}") for h in range(H)]
        v_aug = persist.tile([128, 8, H, F + 1], BF, tag="va")
        catU = [persist.tile([128, S], BF, tag=f"cu{c}", name=f"cu{c}") for c in range(6)]
        wo_h = [persist.tile([128, O], BF, tag=f"wo{c}", name=f"wo{c}") for c in range(6)]
        bo_sb = persist.tile([128, O], F32, tag="bo")
        ones64 = persist.tile([64, 1], BF, tag="o64")
        ones64k = persist.tile([64, 1], BF, tag="o64k")
        ones_r = persist.tile([65, 64], F32, tag="o1")
        ones_row = persist.tile([1, S], BF, tag="onesrow")

        for c in range(6):
            nc.scalar.dma_start(out=wo_h[c], in_=wot[c * 128:(c + 1) * 128, :])
        nc.scalar.dma_start(
            out=bo_sb, in_=bass.AP(tensor=bo, offset=0, ap=[[0, 128], [1, O]])
        )
        nc.gpsimd.memset(ones64, 1.0)
        nc.gpsimd.memset(ones64k, 0.25)
        nc.gpsimd.memset(ones_r[64:65, :], 1.0)
        nc.gpsimd.memset(v_aug[:, :, :, F:F + 1], 1.0)
        nc.gpsimd.memset(ones_row, 1.0)
        for h in range(H):
            # DVE/gpsimd can't start at partition 65; write via DMA
            nc.sync.dma_start(out=q_aug[h][65:66, :], in_=ones_row)
            nc.gpsimd.memset(k_aug[h][64:65, :], 1.0)

        smalls = ctx.enter_context(tc.tile_pool(name="sm", bufs=3))
        opool = ctx.enter_context(tc.tile_pool(name="ob", bufs=3))

        # ---------------- phases 1-3, interleaved emission ----------------
        # The Tile scheduler dispatches per-engine in program order, so
        # attention units are emitted as soon as their head-pair projections
        # exist; ACT starts sqrt/exp work ~25us in instead of ~100us.
        with ExitStack() as p1:
            ld = p1.enter_context(tc.tile_pool(name="ld", bufs=1))
            x_sb = [ld.tile([128, S], BF, tag=f"x{dc}", name=f"x{dc}") for dc in range(6)]
            wqk_sb = [ld.tile([128, 2 * O], BF, tag=f"wqk{dc}", name=f"wqk{dc}") for dc in range(6)]
            wv_sb = [ld.tile([128, O], BF, tag=f"wv{dc}", name=f"wv{dc}") for dc in range(6)]
            for dc in range(6):
                sl = slice(dc * 128, (dc + 1) * 128)
                nc.scalar.dma_start(out=x_sb[dc], in_=xt[sl, :])
                nc.scalar.dma_start(out=wqk_sb[dc][:, 0:512], in_=wqkt[sl, 0:512])
            for dc in range(6):
                sl = slice(dc * 128, (dc + 1) * 128)
                nc.scalar.dma_start(out=wqk_sb[dc][:, 512:1536], in_=wqkt[sl, 512:1536])
            for dc in range(6):
                sl = slice(dc * 128, (dc + 1) * 128)
                nc.scalar.dma_start(out=wv_sb[dc], in_=wvt[sl, :])

            stage = p1.enter_context(tc.tile_pool(name="stage", bufs=4))
            sqp = p1.enter_context(tc.tile_pool(name="sqp", bufs=3))
            nstage = p1.enter_context(tc.tile_pool(name="nstage", bufs=2))

            def v_proj(half):
                for tcn in range(8):
                    pv = pB.tile([128, 6, 64], F32, tag="b", name="pv")
                    for dc in range(6):
                        nc.tensor.matmul(
                            pv,
                            x_sb[dc][:, tcn * 128:(tcn + 1) * 128],
                            wv_sb[dc][:, half * 384:(half + 1) * 384],
                            start=(dc == 0),
                            stop=(dc == 5),
                        )
                    nc.vector.tensor_copy(
                        v_aug[:, tcn, half * 6:(half + 1) * 6, 0:F], pv
                    )

            def qk_head(h):
                # fused projection: wqk col block h*128 = [wq_h | -2*wk_h]
                for sc in range(2):
                    ssl = slice(sc * 512, (sc + 1) * 512)
                    ps = pA.tile([128, 512], F32, tag="a", name="pp")
                    for dc in range(6):
                        nc.tensor.matmul(
                            ps,
                            wqk_sb[dc][:, h * 128:(h + 1) * 128],
                            x_sb[dc][:, ssl],
                            start=(dc == 0),
                            stop=(dc == 5),
                        )
                    st = stage.tile([128, 512], BF, tag="st", name="st")
                    nc.vector.tensor_copy(st, ps)
                    nc.sync.dma_start(out=q_aug[h][0:64, ssl], in_=st[0:64, :])
                    nc.sync.dma_start(out=k_aug[h][0:64, ssl], in_=st[64:128, :])
                # row norms: qn -> q_aug row 64, kn -> k_aug row 65
                for (ones_t, dest, row) in ((ones64, q_aug, 64), (ones64k, k_aug, 65)):
                    sq_t = sqp.tile([64, S], BF, tag="sq", name="sq")
                    nc.vector.tensor_mul(sq_t, dest[h][0:64, :], dest[h][0:64, :])
                    nst = nstage.tile([1, 2, 512], BF, tag="nst", name="nst")
                    for sc in range(2):
                        pn = pC.tile([1, 512], F32, tag="c", name="pn")
                        nc.tensor.matmul(
                            pn,
                            ones_t,
                            sq_t[:, sc * 512:(sc + 1) * 512],
                            start=True,
                            stop=True,
                        )
                        nc.vector.tensor_copy(nst[0:1, sc, :], pn)
                    nc.sync.dma_start(
                        out=dest[h][row:row + 1, :], in_=nst[0:1, :, :]
                    )

            def unit_scores(dpool, h, sc):
                """scores + sqrt for one unit; returns dist tile."""
                ssl = slice(sc * 512, (sc + 1) * 512)
                dist = dpool.tile([128, 8, 512], BF, tag="dist", name="dist")
                for grp in range(4):
                    ps = pA.tile([128, 2, 512], F32, tag="a", name="psc")
                    for j in range(2):
                        tci = grp * 2 + j
                        nc.tensor.matmul(
                            ps[:, j, :],
                            k_aug[h][:, tci * 128:(tci + 1) * 128],
                            q_aug[h][:, ssl],
                            start=True,
                            stop=True,
                        )
                    nc.scalar.activation(
                        out=dist[:, grp * 2:grp * 2 + 2, :],
                        in_=ps,
                        func=AF.Sqrt,
                        scale=1.0 / 768.0,
                    )
                return dist

            def unit_tail(h, sc, dist):
                """exp + AV + normalize for one unit (att in-place in dist)."""
                ssl = slice(sc * 512, (sc + 1) * 512)
                att = dist
                nc.scalar.activation(
                    out=att[:, :, :], in_=dist[:, :, :], func=AF.Exp
                )
                pav = pB.tile([65, 512], F32, tag="b", name="pav")
                for tci in range(8):
                    nc.tensor.matmul(
                        pav,
                        v_aug[:, tci, h, :],
                        att[:, tci, :],
                        start=(tci == 0),
                        stop=(tci == 7),
                    )
                r_t = smalls.tile([65, 512], F32, tag="r", name="r")
                nc.vector.reciprocal(r_t[64:65, :], pav[64:65, :])
                prb = pC.tile([64, 512], F32, tag="c", name="prb")
                nc.tensor.matmul(
                    prb, ones_r[64:65, :], r_t[64:65, :], start=True, stop=True
                )
                rb = smalls.tile([64, 512], BF, tag="rb", name="rb")
                nc.vector.tensor_copy(rb, prb)
                if h % 2 == 0:
                    nc.vector.tensor_mul(catU[h // 2][0:64, ssl], pav[0:64, :], rb)
                else:
                    hm = smalls.tile([64, 512], BF, tag="hm", name="hm")
                    nc.vector.tensor_mul(hm, pav[0:64, :], rb)
                    nc.sync.dma_start(out=catU[h // 2][64:128, ssl], in_=hm)

            def unit_group(dpool, batch, jobs=None, pre_tail=None):
                dists = [unit_scores(dpool, h, sc) for (h, sc) in batch]
                if pre_tail is not None:
                    pre_tail()
                for (h, sc), dist in zip(batch, dists):
                    unit_tail(h, sc, dist)
                    if jobs:
                        out_job(*jobs.pop(0))

            def out_job(si, oc):
                osl = slice(oc * 384, (oc + 1) * 384)
                po = pB.tile([128, 384], F32, tag="b", name="po")
                for c in range(6):
                    nc.tensor.matmul(
                        po,
                        catU[c][:, si * 128:(si + 1) * 128],
                        wo_h[c][:, osl],
                        start=(c == 0),
                        stop=(c == 5),
                    )
                ob = opool.tile([128, 384], F32, tag="ob", name="ob")
                nc.vector.tensor_add(ob, po, bo_sb[:, osl])
                nc.sync.dma_start(out=out[si * 128:(si + 1) * 128, osl], in_=ob)

            def out_linear(si_range):
                for si in si_range:
                    for oc in range(2):
                        out_job(si, oc)

            # interleaved emission: sc0 units as soon as their pair is ready
            with ExitStack() as pe_stack:
                dist_early = pe_stack.enter_context(
                    tc.tile_pool(name="dist_e", bufs=6)
                )
                for h in range(4):
                    qk_head(h)
                unit_group(
                    dist_early,
                    [(0, 0), (1, 0), (2, 0), (3, 0)],
                    pre_tail=lambda: v_proj(0),
                )
                for h in range(4, 8):
                    qk_head(h)
                unit_group(
                    dist_early,
                    [(4, 0), (5, 0), (6, 0), (7, 0)],
                    pre_tail=lambda: v_proj(1),
                )
                for h in range(8, 12):
                    qk_head(h)
                unit_group(dist_early, [(8, 0), (9, 0), (10, 0), (11, 0)])

        # ---------------- sc1 units + output linear ----------------
        for _rep in range(REPEAT):
            with ExitStack() as p2:
                dist_late = p2.enter_context(
                    tc.tile_pool(name="dist_l", bufs=GROUP + 1)
                )
                jobs0 = [(si, oc) for si in range(0, 4) for oc in range(2)]
                unit_group(dist_late, [(h, 1) for h in range(8)], jobs=jobs0)
                unit_group(dist_late, [(h, 1) for h in range(8, 12)], jobs=jobs0)
                out_linear(range(4, 8))


def build_program():
    nc = bacc.Bacc(
        "TRN2", target_bir_lowering=False, debug=False, num_devices=NCORES
    )
    xt = nc.dram_tensor("xt", [D, S], BF, kind="ExternalInput")
    wqkt = nc.dram_tensor("wqkt", [D, 2 * O], BF, kind="ExternalInput")
    wvt = nc.dram_tensor("wvt", [D, O], BF, kind="ExternalInput")
    wot = nc.dram_tensor("wot", [O, O], BF, kind="ExternalInput")
    bo = nc.dram_tensor("bo", [O], F32, kind="ExternalInput")
    out = nc.dram_tensor("out", [S, O], F32, kind="ExternalOutput")
    with tile.TileContext(nc) as tc:
        _build(tc, xt, wqkt, wvt, wot, bo, out)
    nc.compile()
    return nc


_prog = None


def get_program():
    global _prog
    if _prog is None:
        _prog = build_program()
    return _prog


def make_in_maps(inputs):
    bf = ml_dtypes.bfloat16
    x = np.asarray(inputs["x"], np.float32)
    wq = np.asarray(inputs["wq"], np.float32)
    wk = np.asarray(inputs["wk"], np.float32)
    wv = np.asarray(inputs["wv"], np.float32)
    wo = np.asarray(inputs["wo"], np.float32)
    bo = np.ascontiguousarray(np.asarray(inputs["bo"], np.float32))
    wq_r = wq.transpose(1, 0, 2)            # [D, H, F]
    wk2_r = (-2.0 * wk).transpose(1, 0, 2)  # [D, H, F]
    wqkt = np.ascontiguousarray(
        np.stack([wq_r, wk2_r], axis=2).reshape(D, 2 * O)
    ).astype(bf)
    wvt = np.ascontiguousarray(wv.transpose(1, 0, 2).reshape(D, O)).astype(bf)
    wot = np.ascontiguousarray(wo.T).astype(bf)
    in_maps = []
    for c in range(NCORES):
        xtc = np.ascontiguousarray(x[c].T).astype(bf)
        in_maps.append(
            dict(xt=xtc, wqkt=wqkt, wvt=wvt, wot=wot, bo=bo)
        )
    return in_maps


def kernel(**inputs):
    in_maps = make_in_maps(inputs)
    nc = get_program()
    res = run_bass_kernel_spmd(nc, in_maps, list(range(NCORES))).results
    return np.stack(
        [res[c]["out"] for c in range(NCORES)], axis=0
    ).astype(np.float32)
